# revision 1
# baseline (speedup 1.0000x reference)
"""MoE gate kernel (softmax + top-6 routing) for Trainium2, 8-core SPMD.

Computes, for hidden_states [16384, 4096] and gate weight [64, 4096]:
    logits = hidden_states @ weight.T
    scores = softmax(logits)          (top-k selection done on logits --
    topk_weight, topk_idx = top_k(scores, 6)   monotone equivalent)
    topk_weight /= sum(topk_weight)
Returns (topk_idx int32 [16384, 6], topk_weight float32 [16384, 6]).

Sharding: token axis split across 8 cores (2048 tokens each); weight
replicated.  Per core the hidden dim streams through the PE in 32 chunks of
128 with the (transposed) weight stationary, accumulating logits^T [64, T]
in PSUM; logits are PE-transposed back to token-major for the per-token
top-k (DVE max/max_index), and only the 6 winning logits go through exp.
"""

import sys

for _p in ("/root/.axon_site", "/root/.axon_site/_ro/trn_rl_repo",
           "/root/.axon_site/_ro/pypackages", "/opt/trn_rl_repo"):
    if _p not in sys.path:
        sys.path.append(_p)

import numpy as np

N_CORES = 8
N_TOKENS = 16384
HIDDEN = 4096
N_EXPERTS = 64
TOP_K = 6

T_CORE = N_TOKENS // N_CORES          # 2048 tokens per core
CHUNK = 512                           # tokens per pipeline chunk
N_CHUNKS = T_CORE // CHUNK            # 4
KC = HIDDEN // 128                    # 32 k-chunks of 128
KT_PER_DMA = 4                        # k-chunks per 1 MiB DMA
N_KDMA = KC // KT_PER_DMA             # 8 DMAs per chunk
GROUPS = CHUNK // 128                 # 4 transpose groups per chunk

MM_DTYPE = "float32"                  # "float32" | "float32r" for the matmul
DMA_ALT = True                        # alternate x DMAs between sync/scalar
XBUFS = 16                            # x-tile pool depth

_PROGRAM = None


def _build_program(n_iters: int = 1):
    import concourse.bacc as bacc
    import concourse.tile as tile
    import concourse.mybir as mybir
    import concourse.bass as bass
    from concourse import masks

    f32 = mybir.dt.float32
    i32 = mybir.dt.int32
    u32 = mybir.dt.uint32
    mmdt = getattr(mybir.dt, MM_DTYPE)

    nc = bacc.Bacc("TRN2", target_bir_lowering=False, debug=False,
                   num_devices=N_CORES)

    xs_h = nc.dram_tensor("xs", [N_CHUNKS, N_KDMA, 128, KT_PER_DMA, CHUNK],
                          mmdt, kind="ExternalInput")
    wt_h = nc.dram_tensor("wt", [128, KC, N_EXPERTS], mmdt,
                          kind="ExternalInput")
    oi_h = nc.dram_tensor("oidx", [T_CORE, TOP_K], i32, kind="ExternalOutput")
    ow_h = nc.dram_tensor("ow", [T_CORE, TOP_K], f32, kind="ExternalOutput")

    with tile.TileContext(nc) as tc:
        with (
            tc.tile_pool(name="const", bufs=1) as cpool,
            tc.tile_pool(name="xin", bufs=XBUFS) as xpool,
            tc.tile_pool(name="ps_log", bufs=4, space=bass.MemorySpace.PSUM) as pslog,
            tc.tile_pool(name="ps_w", bufs=1, space=bass.MemorySpace.PSUM) as pswrm,
            tc.tile_pool(name="ps_tr", bufs=3, space=bass.MemorySpace.PSUM) as pstr,
            tc.tile_pool(name="lg", bufs=4) as lgpool,
            tc.tile_pool(name="tk", bufs=4) as tkpool,
        ):
            # Engine warm-up: the first use of PE / ACT-exp / DVE-max each
            # pulls its init or ucode table (~90/86/171 us).  Issued back to
            # back on independent dummy tiles here, the three loads overlap
            # each other and the x stream instead of serializing on the
            # chunk-0 critical path.
            wrm = cpool.tile([128, 16], f32)
            nc.gpsimd.memset(wrm[:], 0.0)
            wrm_ps = pswrm.tile([16, 16], f32)
            nc.tensor.matmul(wrm_ps[:], wrm[:, 0:16], wrm[:])
            wrm_e = cpool.tile([128, 16], f32)
            nc.scalar.activation(wrm_e[:], wrm[:],
                                 mybir.ActivationFunctionType.Exp)
            wrm_m = cpool.tile([128, 8], f32)
            nc.vector.max(wrm_m[:], wrm[:])
            wrm_i = cpool.tile([128, 8], u32)
            nc.vector.max_index(wrm_i[:], wrm_m[:], wrm[:])

            wt_sb = cpool.tile([128, KC, N_EXPERTS], mmdt)
            nc.gpsimd.dma_start(wt_sb[:], wt_h.ap())
            ident = cpool.tile([64, 64], f32)
            masks.make_identity(nc, ident[:])


            for it in range(n_iters):
                for c in range(N_CHUNKS):
                    ps_l = pslog.tile([N_EXPERTS, CHUNK], f32)
                    for j in range(N_KDMA):
                        xt = xpool.tile([128, KT_PER_DMA, CHUNK], mmdt)
                        eng = nc.sync if ((c * N_KDMA + j) % 2 == 0 or not DMA_ALT) else nc.scalar
                        eng.dma_start(xt[:], xs_h.ap()[c, j])
                        for kt in range(KT_PER_DMA):
                            k = j * KT_PER_DMA + kt
                            nc.tensor.matmul(
                                ps_l[:], wt_sb[:, k, :], xt[:, kt, :],
                                start=(k == 0), stop=(k == KC - 1),
                            )
                    # logits^T [64, CHUNK] -> sbuf
                    tl = lgpool.tile([N_EXPERTS, CHUNK], f32)
                    nc.scalar.activation(tl[:], ps_l[:],
                                         mybir.ActivationFunctionType.Copy)
                    # transpose to token-major [128, 64] per 128-token group
                    ps_t = pstr.tile([128, GROUPS, N_EXPERTS], f32)
                    sc = lgpool.tile([128, GROUPS, N_EXPERTS], f32)
                    for g in range(GROUPS):
                        nc.tensor.transpose(ps_t[:, g, :],
                                            tl[:, g * 128:(g + 1) * 128],
                                            ident[:])
                        nc.vector.tensor_copy(sc[:, g, :], ps_t[:, g, :])
                    # top-8 values + indices per token (on logits)
                    l8 = tkpool.tile([128, GROUPS, 8], f32)
                    ix8 = tkpool.tile([128, GROUPS, 8], u32)
                    for g in range(GROUPS):
                        nc.vector.max(l8[:, g, :], sc[:, g, :])
                        nc.vector.max_index(ix8[:, g, :], l8[:, g, :],
                                            sc[:, g, :])
                    # index output can ship as soon as max_index is done
                    off = c * CHUNK * TOP_K
                    pat = [[TOP_K, 128], [128 * TOP_K, GROUPS], [1, TOP_K]]
                    i6 = tkpool.tile([128, GROUPS, TOP_K], i32)
                    nc.vector.tensor_copy(i6[:], ix8[:, :, 0:TOP_K])
                    oeng = nc.sync if c == N_CHUNKS - 1 else nc.gpsimd
                    oeng.dma_start(bass.AP(oi_h, off, pat), i6[:])
                    # weights: exp(l_j - l_max) of the 6 winners, normalized.
                    # Subtracting the row max (= l8[...,0]) reproduces the
                    # reference softmax's exp arguments bit-exactly, so the
                    # ACT table error cancels against the reference.
                    negm = tkpool.tile([128, GROUPS], f32)
                    nc.vector.tensor_scalar_mul(negm[:], l8[:, :, 0], -1.0)
                    e6 = tkpool.tile([128, GROUPS, TOP_K], f32)
                    for g in range(GROUPS):
                        nc.scalar.activation(e6[:, g, :], l8[:, g, 0:TOP_K],
                                             mybir.ActivationFunctionType.Exp,
                                             bias=negm[:, g:g + 1])
                    den = tkpool.tile([128, GROUPS], f32)
                    nc.vector.reduce_sum(den[:], e6[:],
                                         axis=mybir.AxisListType.X)
                    rec = tkpool.tile([128, GROUPS], f32)
                    nc.vector.reciprocal(rec[:], den[:])
                    w6 = tkpool.tile([128, GROUPS, TOP_K], f32)
                    nc.vector.tensor_mul(
                        w6[:], e6[:],
                        rec[:].unsqueeze(2).broadcast_to((128, GROUPS, TOP_K)))
                    oeng.dma_start(bass.AP(ow_h, off, pat), w6[:])

    nc.compile()
    return nc


def _get_program():
    global _PROGRAM
    if _PROGRAM is None:
        _PROGRAM = _build_program(1)
    return _PROGRAM


def _prep_inputs(hidden_states: np.ndarray, weight: np.ndarray):
    """Build per-core input maps (token-sharded x, replicated weight)."""
    w = np.ascontiguousarray(weight.astype(np.float32, copy=False))
    # wt[p, k, e] = W[e, k*128 + p]
    wt = np.ascontiguousarray(w.T.reshape(KC, 128, N_EXPERTS)
                              .transpose(1, 0, 2))
    in_maps = []
    for cid in range(N_CORES):
        shard = hidden_states[cid * T_CORE:(cid + 1) * T_CORE]
        # xs[c, j, p, kt, t] = shard[c*512 + t, (j*4 + kt)*128 + p]
        xs = (shard.astype(np.float32, copy=False).T
              .reshape(N_KDMA, KT_PER_DMA, 128, N_CHUNKS, CHUNK)
              .transpose(3, 0, 2, 1, 4))
        in_maps.append({"xs": np.ascontiguousarray(xs), "wt": wt})
    return in_maps


def kernel(hidden_states: np.ndarray, weight: np.ndarray):
    from concourse.bass_utils import run_bass_kernel_spmd

    hidden_states = np.asarray(hidden_states)
    weight = np.asarray(weight)
    nc = _get_program()
    in_maps = _prep_inputs(hidden_states, weight)
    res = run_bass_kernel_spmd(nc, in_maps, list(range(N_CORES)),
                               trace=False)
    idx = np.concatenate([res.results[i]["oidx"] for i in range(N_CORES)],
                         axis=0)
    wgt = np.concatenate([res.results[i]["ow"] for i in range(N_CORES)],
                         axis=0)
    return idx.astype(np.int32, copy=False), wgt.astype(np.float32, copy=False)

